# revision 47
# baseline (speedup 1.0000x reference)
"""Self-contained Trainium2 Bass kernel for nn_MultiHeadAttention_80942953660675.

Reference computation (B=2, T=2048, D=1024, H=16, hd=64, causal):
    q = x @ wq.T; k = x @ wk.T; v = x @ wv.T            (per-head split)
    out = softmax(q k^T / sqrt(hd) + causal_mask) v      (per batch, head)
    out = concat_heads(out) @ wo.T + bo

Sharding over 8 NeuronCores: core = (batch b, head-group g), b in {0,1},
g in {0..3}, each group = 4 heads (256 channels). wq/wk/wv column-sharded,
wo row-sharded (Megatron); host sums the 4 partial outputs per batch and
adds the bias.

Per-core kernel, v2 (all-bf16 data path, fp32 PSUM accumulation):
  - xT persistent in SBUF (4 MB bf16); weights bf16 (FWL doubles weight
    load rate vs fp32, which is what lets the packed scores run 2x)
  - scores: 2 heads packed per slot as concurrent K=64 row-tiled matmuls
    (tile_position (0,0)/(64,0) via base_partition), each [128kc x 512q]
    into adjacent PSUM banks -> one [128,1024] exp per slot
  - softmax without max subtraction (scores O(+-6)); denominator = ones
    column appended to V; normalization: per-head ACT Reciprocal on the
    [1,512] denominator row, broadcast to 64 partitions with a K=1
    f32r matmul, one DVE multiply writes AT in bf16
  - causal: above-diagonal kc tiles skipped, diagonal tiles masked after
    exp (alternating DVE mask-multiply / gpsimd affine_select)
  - global software pipeline: projection groups of block tb+1 and output-
    projection groups of block qb-1 are interleaved into stage B's
    exp-paced stream so the PE never idles; K/V projections of the last
    block are deferred into stage B(3) to fill its ACT-bound bubble
"""

import sys
import types

if "/opt/trn_rl_repo" not in sys.path:
    sys.path.insert(0, "/opt/trn_rl_repo")

import numpy as np

B, T, D = 2, 2048, 1024
H, HD = 16, 64
NCORES = 8
GROUPS = 4            # head groups (cores per batch)
HPC = H // GROUPS     # heads per core = 4
CH = HPC * HD         # channels per core = 256

NDC = D // 128        # 8   d-chunks (contraction for projections)
NCC = CH // 128       # 2   channel chunks = head pairs
NQB = T // 512        # 4   query blocks
NKC = T // 128        # 16  key chunks
NTC = T // 128        # 16  token chunks
NEB = D // 512        # 2   embed blocks (output projection)


def _install_axon_ntff_hook():
    """Inject the missing antenv.axon_hooks module so NTFF profiling
    (trace=True) works in this container. Harmless if never used."""
    if "antenv.axon_hooks" in sys.modules:
        return
    try:
        import antenv  # noqa: F401
    except ImportError:
        return
    mod = types.ModuleType("antenv.axon_hooks")
    mod._hook = None

    def _set(h):
        mod._hook = h

    def _get():
        return mod._hook

    mod.set_axon_ntff_profile_hook = _set
    mod.get_axon_ntff_profile_hook = _get
    sys.modules["antenv.axon_hooks"] = mod
    try:
        from trn_agent_boot.trn_boot import _ntff_profile_via_ctypes

        _set(_ntff_profile_via_ctypes("/opt/axon/libaxon_pjrt.so"))
    except Exception:
        pass


def _patch_tile_drain():
    """This walrus build rejects >2 embedded sync waits on a single
    instruction; TileContext's exit drain can carry many. Split the extras
    onto nop instructions placed just before the drain."""
    import concourse.tile as tile

    if getattr(tile.TileContext, "_drain_split_patched", False):
        return
    import bass_rust as _br
    from concourse.vector_clock import ScopedClock as _ScopedClock

    def _split_drain_and_barrier(self, tick_clock, wait_clock):
        nc = self.nc
        drain_inst = nc.sync.drain()
        wait_clock.add_sem_waits(
            drain_inst.ins, _ScopedClock({None: tick_clock.global_clock})
        )
        si = drain_inst.ins.sync_info
        waits = list(si.on_wait) if (si is not None and si.on_wait) else []
        if len(waits) > 1:
            bb = nc.cur_bb.bb
            si.on_wait = waits[:1]
            new_insts = []
            for w in waits[1:]:
                nop = nc.sync.nop()
                nop.ins.sync_info = _br.SyncInfo(on_wait=[w], on_update=[])
                bb.instructions.remove(nop.ins)
                new_insts.append(nop.ins)
            idx = bb.instructions.index(drain_inst.ins)
            for ni in reversed(new_insts):
                bb.instructions.insert(idx, ni)

        nc.all_engine_barrier()
        assert self.sems is not None
        popped = nc._tile_sem_poison_stack.pop()
        assert popped is self._sem_poison
        nc.clear_and_free_semaphores(list(self.sems.allocated().values()))
        nc.all_engine_barrier()

    tile.TileContext._drain_and_barrier = _split_drain_and_barrier
    tile.TileContext._drain_split_patched = True


def build_nc(causal: bool):
    """Build the SPMD Bass program (identical on all 8 cores)."""
    _patch_tile_drain()
    from contextlib import ExitStack

    import concourse.bacc as bacc
    import concourse.tile as tile
    from concourse import mybir

    f32 = mybir.dt.float32
    f32r = mybir.dt.float32r
    bf16 = mybir.dt.bfloat16
    Exp = mybir.ActivationFunctionType.Exp
    Ln = mybir.ActivationFunctionType.Ln

    nc = bacc.Bacc("TRN2")
    # all inputs host-rearranged to partition-major so every DMA moves
    # multi-KB contiguous runs per partition (descriptor-count, not bytes,
    # is what throttles the DMA ring)
    xT_d = nc.dram_tensor("xT", [128, NQB, NDC, 512], bf16, kind="ExternalInput")
    wq_d = nc.dram_tensor("wq", [128, NDC, CH], bf16, kind="ExternalInput")
    wk_d = nc.dram_tensor("wk", [128, NDC, CH], bf16, kind="ExternalInput")
    wv_d = nc.dram_tensor("wv", [128, NDC, CH], bf16, kind="ExternalInput")
    wo_d = nc.dram_tensor("wo", [128, NCC, D], bf16, kind="ExternalInput")
    sel_d = nc.dram_tensor("sel", [4, HPC * 64], f32r, kind="ExternalInput")
    out_d = nc.dram_tensor("out", [T, D], bf16, kind="ExternalOutput")

    with tile.TileContext(nc) as tc:
        with ExitStack() as ctx:
            persist = ctx.enter_context(tc.tile_pool(name="persist", bufs=1))
            mm_ps = ctx.enter_context(
                tc.tile_pool(name="mm_ps", bufs=2, space="PSUM")
            )
            s_ps = ctx.enter_context(tc.tile_pool(name="s_ps", bufs=2, space="PSUM"))
            pv_ps = ctx.enter_context(tc.tile_pool(name="pv_ps", bufs=2, space="PSUM"))
            p_pool = ctx.enter_context(tc.tile_pool(name="p_pool", bufs=6))
            rc_pool = ctx.enter_context(tc.tile_pool(name="rc_pool", bufs=4))
            pvs_pool = ctx.enter_context(tc.tile_pool(name="pvs_pool", bufs=6))
            ob_pool = ctx.enter_context(tc.tile_pool(name="ob_pool", bufs=3))

            # ---- persistent SBUF tensors ----
            xT_sb = persist.tile([128, NQB, NDC, 512], bf16, tag="xT")  # 4 MB
            wq_sb = persist.tile([128, NDC, CH], bf16, tag="wq")       # 0.5 MB
            wk_sb = persist.tile([128, NDC, CH], bf16, tag="wk")
            wv_sb = persist.tile([128, NDC, CH], bf16, tag="wv")
            wo_sb = persist.tile([128, NCC, D], bf16, tag="wo")        # 0.5 MB
            QT_sb = persist.tile([128, NCC, T], bf16, tag="QT")        # 1 MB
            KT_sb = persist.tile([128, NCC, T], bf16, tag="KT")        # 1 MB
            V_sb = persist.tile([128, NTC, HPC, HD + 1], bf16, tag="V")
            AT_sb = persist.tile([128, NCC, T], bf16, tag="AT")
            maskm = persist.tile([128, 4, 1024], bf16, tag="maskm")
            # head-selector for the denominator broadcast matmul:
            # sel4[p, h*64+j] = (p == h), so sel4[:, h*64:(h+1)*64].T @ rc4
            # replicates rc4 row h onto 64 partitions
            sel4 = persist.tile([4, HPC * 64], f32r, tag="sel4")
            # per-(head, qb) softmax denominators, gathered cross-partition
            # by tiny SBUF->SBUF DMAs so one reciprocal covers 4 heads
            den4 = persist.tile([4, NQB, 512], f32, tag="den4")

            # ---- input DMAs, ordered so stage A(0) can start ASAP; one
            # issue per tensor/block (each dma_start serializes ~0.6us on
            # the Sync queue, so fewer+bigger is strictly better) ----
            nc.sync.dma_start(wq_sb[:], wq_d[:])
            nc.sync.dma_start(sel4[:], sel_d[:])
            nc.sync.dma_start(xT_sb[:, 0], xT_d[:, 0])
            nc.sync.dma_start(wk_sb[:], wk_d[:])
            nc.sync.dma_start(wv_sb[:], wv_d[:])
            nc.sync.dma_start(xT_sb[:, 1], xT_d[:, 1])
            nc.sync.dma_start(wo_sb[:], wo_d[:])
            nc.sync.dma_start(xT_sb[:, 2], xT_d[:, 2])
            nc.sync.dma_start(xT_sb[:, 3], xT_d[:, 3])

            # ones column of V (softmax denominator trick) — memset, a DMA
            # of this strided pattern shatters into 8192 2-byte descriptors
            nc.vector.memset(V_sb[:, :, :, HD : HD + 1], 1.0)
            # 0/1 causal masks for the four diagonal-kc offsets (i = kc-4qb);
            # used by the DVE mask path (gpsimd affine_select handles the
            # alternating halves of the stream).
            nc.vector.memset(maskm[:], 1.0)
            for i in range(4):
                nc.gpsimd.affine_select(
                    out=maskm[:, i, :].rearrange("p (a b) -> p a b", a=2),
                    in_=maskm[:, i, :].rearrange("p (a b) -> p a b", a=2),
                    compare_op=mybir.AluOpType.is_ge,
                    fill=0.0,
                    base=-128 * i,
                    pattern=[[0, 2], [1, 512]],
                    channel_multiplier=-1,
                )

            # ---- stage A: one projection "group" = one PSUM accumulation ----
            def a_group(tb, kind, j):
                tsl = slice(tb * 512, (tb + 1) * 512)
                if kind in ("Q", "K"):
                    w_sb, dst = (wq_sb, QT_sb) if kind == "Q" else (wk_sb, KT_sb)
                    ps = mm_ps.tile([128, 512], f32, tag="mmps")
                    for dc in range(NDC):
                        nc.tensor.matmul(
                            ps[:],
                            w_sb[:, dc, j * 128 : (j + 1) * 128],
                            xT_sb[:, tb, dc, :],
                            start=(dc == 0),
                            stop=(dc == NDC - 1),
                        )
                    nc.vector.tensor_copy(dst[:, j, tsl], ps[:])
                else:  # V
                    t_c = tb * 4 + j
                    ps = mm_ps.tile([128, 512], f32, tag="mmps")
                    for dc in range(NDC):
                        nc.tensor.matmul(
                            ps[:, 0:CH],
                            xT_sb[:, tb, dc, j * 128 : (j + 1) * 128],
                            wv_sb[:, dc, :],
                            start=(dc == 0),
                            stop=(dc == NDC - 1),
                        )
                    nc.vector.tensor_copy(
                        V_sb[:, t_c, :, 0:HD],
                        ps[:, 0:CH].rearrange("p (h d) -> p h d", h=HPC),
                    )

            def a_groups(tb, kinds="QKV"):
                out = []
                if "Q" in kinds:
                    out += [("A", tb, "Q", j) for j in range(NCC)]
                if "K" in kinds:
                    out += [("A", tb, "K", j) for j in range(NCC)]
                if "V" in kinds:
                    out += [("A", tb, "V", j) for j in range(4)]
                return out

            # ---- stage C: one group = one output token-chunk (2 psums,
            # one merged 256KB store) ----
            def c_group(qb, t_ci):
                t_c = qb * 4 + t_ci
                ob = ob_pool.tile([128, 1024], bf16, tag="ob")
                for eb in range(NEB):
                    esl = slice(eb * 512, (eb + 1) * 512)
                    ps = mm_ps.tile([128, 512], f32, tag="mmps")
                    for cc in range(NCC):
                        nc.tensor.matmul(
                            ps[:],
                            AT_sb[:, cc, t_c * 128 : (t_c + 1) * 128],
                            wo_sb[:, cc, esl],
                            start=(cc == 0),
                            stop=(cc == NCC - 1),
                        )
                    nc.vector.tensor_copy(ob[:, esl], ps[:])
                nc.sync.dma_start(out_d[t_c * 128 : (t_c + 1) * 128, :], ob[:])

            def emit_filler(f):
                if f[0] == "A":
                    a_group(f[1], f[2], f[3])
                elif f[0] == "N":
                    norm_head(f[1], f[2])
                else:
                    c_group(f[1], f[2])

            # ---- stage B ----
            state = {"mask_flip": False, "u": 0}

            def emit_unit(qb, p, kc):
                """Scores for heads (2p, 2p+1) on key-chunk kc: two
                concurrent K=64 row-tiled matmuls -> [128,1024] psum,
                one exp, optional diagonal mask."""
                qsl = slice(qb * 512, (qb + 1) * 512)
                ksl = slice(kc * 128, (kc + 1) * 128)
                s = s_ps.tile([128, 1024], f32, tag="s")
                nc.tensor.matmul(
                    s[:, 0:512],
                    KT_sb[0:64, p, ksl],
                    QT_sb[0:64, p, qsl],
                    start=True,
                    stop=True,
                )
                nc.tensor.matmul(
                    s[:, 512:1024],
                    KT_sb[64:128, p, ksl],
                    QT_sb[64:128, p, qsl],
                    start=True,
                    stop=True,
                )
                pt = p_pool.tile([128, 1024], bf16, tag="p")
                nc.scalar.activation(pt[:], s[:], Exp)
                if causal and kc >= 4 * qb:
                    i = kc - 4 * qb
                    state["mask_flip"] = not state["mask_flip"]
                    if state["mask_flip"]:
                        nc.vector.tensor_mul(pt[:], pt[:], maskm[:, i, :])
                    else:
                        nc.gpsimd.affine_select(
                            out=pt[:].rearrange("p (a b) -> p a b", a=2),
                            in_=pt[:].rearrange("p (a b) -> p a b", a=2),
                            compare_op=mybir.AluOpType.is_ge,
                            fill=0.0,
                            base=-128 * i,
                            pattern=[[0, 2], [1, 512]],
                            channel_multiplier=-1,
                        )
                return pt

            def finish_block(qb, p, pv0, pv1):
                """Evacuate the two PV accumulators of block (qb, p) and
                ship their denominator rows into den4."""
                for hoi, pv in ((0, pv0), (1, pv1)):
                    h = 2 * p + hoi
                    pvs = pvs_pool.tile([HD + 1, 512], f32, name="pvs", tag="pvs")
                    nc.vector.tensor_copy(pvs[:], pv[:])
                    nc.sync.dma_start(den4[h : h + 1, qb, :], pvs[HD : HD + 1, :])
                    state[("pvs", qb, h)] = pvs

            def emit_recip(qb):
                """Reciprocal of all 4 heads' denominators as exp(-ln(x)) on
                the ACT engine (~1.4us vs 3.3us for the DVE multi-pass
                reciprocal, and off the busier DVE queue); the per-head
                broadcast+normalize is deferred into the filler stream
                (norm_head) so the PE never waits on it."""
                ln4 = rc_pool.tile([4, 512], f32, name="ln4", tag="rc")
                nc.scalar.activation(ln4[:], den4[:, qb, :], Ln)
                rc4 = rc_pool.tile([4, 512], f32r, name="rc4", tag="rc")
                nc.scalar.activation(rc4[:], ln4[:], Exp, scale=-1.0)
                state[("rc", qb)] = rc4

            def norm_head(qb, h):
                qsl = slice(qb * 512, (qb + 1) * 512)
                dn = mm_ps.tile([64, 512], f32, name="dn", tag="mmps")
                nc.tensor.matmul(
                    dn[:],
                    sel4[:, h * 64 : (h + 1) * 64],
                    state[("rc", qb)][:],
                    start=True,
                    stop=True,
                )
                pvs = state.pop(("pvs", qb, h))
                nc.vector.tensor_mul(
                    AT_sb[(h % 2) * 64 : (h % 2 + 1) * 64, h // 2, qsl],
                    pvs[0:HD, :],
                    dn[:],
                )

            pend_pv = []
            done_blocks = {qb: 0 for qb in range(NQB)}
            flow = []  # filler queue consumed by the unit loop

            def pop_pv():
                qb, p, kc, last, pt, pv0, pv1 = pend_pv.pop(0)
                nc.tensor.matmul(
                    pv0[:],
                    V_sb[:, kc, 2 * p, :],
                    pt[:, 0:512],
                    start=(kc == 0),
                    stop=last,
                )
                nc.tensor.matmul(
                    pv1[:],
                    V_sb[:, kc, 2 * p + 1, :],
                    pt[:, 512:1024],
                    start=(kc == 0),
                    stop=last,
                )
                if last:
                    finish_block(qb, p, pv0, pv1)
                    done_blocks[qb] += 1
                    if done_blocks[qb] == NCC:
                        emit_recip(qb)
                        state[("recip_u", qb)] = state["u"]
                        flow.extend(("N", qb, h) for h in range(HPC))
                        flow.extend(("C", qb, t_ci) for t_ci in range(4))

            # ---- emission schedule ----
            # A(0) head; per qb: its B units with interleaved fillers.
            # A(3) is split: Q(3) into B(2) (QT(3) gates B(3) start), K/V(3)
            # into B(3)'s early units (legal for kc<12) to fill its
            # ACT-bound bubble; C(qb) becomes ready mid-stream via pop_pv.
            for f in a_groups(0):
                emit_filler(f)

            if causal:
                section_fillers = {
                    0: a_groups(1),
                    1: a_groups(2),
                    2: a_groups(3, "Q"),
                    3: a_groups(3, "KV"),
                }
            else:
                # every query block attends to every key chunk: all
                # projections must precede stage B
                for tb in range(1, NQB):
                    for f in a_groups(tb):
                        emit_filler(f)
                section_fillers = {qb: [] for qb in range(NQB)}

            def drip(hold_c, final=False):
                """Emit one filler. Skips stage-C groups while they are held
                back to cover the tail's norm chain, and norm_head entries
                until the recip (queued on ACT behind pending exps) has had
                ~3 units to complete, so the dn matmul never stalls the PE."""
                for idx, f in enumerate(flow):
                    if not final:
                        if hold_c and f[0] == "C":
                            continue
                        if (
                            f[0] == "N"
                            and state["u"] - state[("recip_u", f[1])] < 3
                        ):
                            continue
                        if f[0] == "C" and any(
                            g[0] == "N" and g[1] == f[1] for g in flow[:idx]
                        ):
                            continue  # keep C(qb) after all N(qb)
                    emit_filler(flow.pop(idx))
                    return True
                return False

            for qb in range(NQB):
                nkc = 4 * (qb + 1) if causal else NKC
                flow.extend(section_fillers[qb])
                units = [(p, kc) for p in range(NCC) for kc in range(nkc)]
                hold_c = qb == NQB - 1
                for ui, (p, kc) in enumerate(units):
                    # correctness guard: this unit's K/V block must be
                    # projected already (only B(3)'s deferred K/V(3) can hit)
                    while any(
                        f[0] == "A" and f[1] <= kc // 4 for f in flow
                    ):
                        emit_filler(flow.pop(0))
                    if kc == 0:
                        state["pv"] = (
                            pv_ps.tile([HD + 1, 512], f32, name="pv0", tag="pv"),
                            pv_ps.tile([HD + 1, 512], f32, name="pv1", tag="pv"),
                        )
                    pv0, pv1 = state["pv"]
                    state["u"] += 1
                    pt = emit_unit(qb, p, kc)
                    pend_pv.append((qb, p, kc, kc == nkc - 1, pt, pv0, pv1))
                    if len(pend_pv) > 3:
                        pop_pv()
                    drip(hold_c)
                # A fillers gate the next section; flush them now
                while any(f[0] == "A" for f in flow):
                    emit_filler(flow.pop(0))

            while pend_pv:
                pop_pv()
            while flow:
                emit_filler(flow.pop(0))

    nc.finalize()
    return nc


def make_in_maps(q_input, wq, wk, wv, wo):
    import ml_dtypes

    bf16 = ml_dtypes.bfloat16
    q_input = np.asarray(q_input, dtype=np.float32)
    wq = np.asarray(wq, dtype=np.float32)
    wk = np.asarray(wk, dtype=np.float32)
    wv = np.asarray(wv, dtype=np.float32)
    wo = np.asarray(wo, dtype=np.float32)
    scale = 1.0 / np.sqrt(np.float32(HD))
    sel = np.zeros((4, HPC * 64), np.float32)
    for h in range(HPC):
        sel[h, h * 64 : (h + 1) * 64] = 1.0

    def dmajor(w):  # [D, c] -> [128, NDC, c] partition-major
        return np.ascontiguousarray(
            w.reshape(NDC, 128, w.shape[1]).transpose(1, 0, 2)
        ).astype(bf16)

    in_maps = []
    for core in range(NCORES):
        b, g = divmod(core, GROUPS)
        G = slice(g * CH, (g + 1) * CH)
        xT = q_input[b].T  # [D, T]
        xT = np.ascontiguousarray(
            xT.reshape(NDC, 128, NQB, 512).transpose(1, 2, 0, 3)
        ).astype(bf16)  # [128, NQB, NDC, 512]
        wo_r = wo[:, G].T  # [CH, D]
        wo_r = np.ascontiguousarray(
            wo_r.reshape(NCC, 128, D).transpose(1, 0, 2)
        ).astype(bf16)  # [128, NCC, D]
        in_maps.append(
            {
                "xT": xT,
                "wq": dmajor(wq[G, :].T * scale),
                "wk": dmajor(wk[G, :].T),
                "wv": dmajor(wv[G, :].T),
                "wo": wo_r,
                "sel": sel,
            }
        )
    return in_maps


def _gather(results, bo):
    out = np.zeros((B, T, D), np.float32)
    for core in range(NCORES):
        out[core // GROUPS] += np.asarray(results[core]["out"], dtype=np.float32)
    out += np.asarray(bo, dtype=np.float32)
    return out


def _run(q_input, wq, wk, wv, wo, bo, mask, trace=False, trace_kwargs=None):
    _install_axon_ntff_hook()
    from concourse.bass_utils import run_bass_kernel_spmd

    causal = bool(np.asarray(mask).item()) if not isinstance(mask, int) else bool(mask)
    nc = build_nc(causal)
    in_maps = make_in_maps(q_input, wq, wk, wv, wo)
    res = run_bass_kernel_spmd(
        nc,
        in_maps,
        list(range(NCORES)),
        trace=trace,
        **(trace_kwargs or {}),
    )
    return _gather(res.results, bo), res


def kernel(q_input, wq, wk, wv, wo, bo, mask):
    out, _ = _run(q_input, wq, wk, wv, wo, bo, mask)
    return out


# revision 51
# speedup vs baseline: 1.1819x; 1.1819x over previous
"""Self-contained Trainium2 Bass kernel for nn_MultiHeadAttention_80942953660675.

Reference computation (B=2, T=2048, D=1024, H=16, hd=64, causal):
    q = x @ wq.T; k = x @ wk.T; v = x @ wv.T            (per-head split)
    out = softmax(q k^T / sqrt(hd) + causal_mask) v      (per batch, head)
    out = concat_heads(out) @ wo.T + bo

Sharding over 8 NeuronCores: core = (batch b, head-group g), b in {0,1},
g in {0..3}, each group = 4 heads (256 channels). wq/wk/wv column-sharded,
wo row-sharded (Megatron); host sums the 4 partial outputs per batch and
adds the bias.

Per-core kernel, v2 (all-bf16 data path, fp32 PSUM accumulation):
  - xT persistent in SBUF (4 MB bf16); weights bf16 (FWL doubles weight
    load rate vs fp32, which is what lets the packed scores run 2x)
  - scores: 2 heads packed per slot as concurrent K=64 row-tiled matmuls
    (tile_position (0,0)/(64,0) via base_partition), each [128kc x 512q]
    into adjacent PSUM banks -> one [128,1024] exp per slot
  - softmax without max subtraction (scores O(+-6)); denominator = ones
    column appended to V; normalization: per-head ACT Reciprocal on the
    [1,512] denominator row, broadcast to 64 partitions with a K=1
    f32r matmul, one DVE multiply writes AT in bf16
  - causal: above-diagonal kc tiles skipped, diagonal tiles masked after
    exp (alternating DVE mask-multiply / gpsimd affine_select)
  - global software pipeline: projection groups of block tb+1 and output-
    projection groups of block qb-1 are interleaved into stage B's
    exp-paced stream so the PE never idles; K/V projections of the last
    block are deferred into stage B(3) to fill its ACT-bound bubble
"""

import sys
import types

if "/opt/trn_rl_repo" not in sys.path:
    sys.path.insert(0, "/opt/trn_rl_repo")

import numpy as np

B, T, D = 2, 2048, 1024
H, HD = 16, 64
NCORES = 8
GROUPS = 4            # head groups (cores per batch)
HPC = H // GROUPS     # heads per core = 4
CH = HPC * HD         # channels per core = 256

NDC = D // 128        # 8   d-chunks (contraction for projections)
NCC = CH // 128       # 2   channel chunks = head pairs
NQB = T // 512        # 4   query blocks
NKC = T // 128        # 16  key chunks
NTC = T // 128        # 16  token chunks
NEB = D // 512        # 2   embed blocks (output projection)


def _install_axon_ntff_hook():
    """Inject the missing antenv.axon_hooks module so NTFF profiling
    (trace=True) works in this container. Harmless if never used."""
    if "antenv.axon_hooks" in sys.modules:
        return
    try:
        import antenv  # noqa: F401
    except ImportError:
        return
    mod = types.ModuleType("antenv.axon_hooks")
    mod._hook = None

    def _set(h):
        mod._hook = h

    def _get():
        return mod._hook

    mod.set_axon_ntff_profile_hook = _set
    mod.get_axon_ntff_profile_hook = _get
    sys.modules["antenv.axon_hooks"] = mod
    try:
        from trn_agent_boot.trn_boot import _ntff_profile_via_ctypes

        _set(_ntff_profile_via_ctypes("/opt/axon/libaxon_pjrt.so"))
    except Exception:
        pass


def _patch_tile_drain():
    """This walrus build rejects >2 embedded sync waits on a single
    instruction; TileContext's exit drain can carry many. Split the extras
    onto nop instructions placed just before the drain."""
    import concourse.tile as tile

    if getattr(tile.TileContext, "_drain_split_patched", False):
        return
    import bass_rust as _br
    from concourse.vector_clock import ScopedClock as _ScopedClock

    def _split_drain_and_barrier(self, tick_clock, wait_clock):
        nc = self.nc
        drain_inst = nc.sync.drain()
        wait_clock.add_sem_waits(
            drain_inst.ins, _ScopedClock({None: tick_clock.global_clock})
        )
        si = drain_inst.ins.sync_info
        waits = list(si.on_wait) if (si is not None and si.on_wait) else []
        if len(waits) > 1:
            bb = nc.cur_bb.bb
            si.on_wait = waits[:1]
            new_insts = []
            for w in waits[1:]:
                nop = nc.sync.nop()
                nop.ins.sync_info = _br.SyncInfo(on_wait=[w], on_update=[])
                bb.instructions.remove(nop.ins)
                new_insts.append(nop.ins)
            idx = bb.instructions.index(drain_inst.ins)
            for ni in reversed(new_insts):
                bb.instructions.insert(idx, ni)

        nc.all_engine_barrier()
        assert self.sems is not None
        popped = nc._tile_sem_poison_stack.pop()
        assert popped is self._sem_poison
        nc.clear_and_free_semaphores(list(self.sems.allocated().values()))
        nc.all_engine_barrier()

    tile.TileContext._drain_and_barrier = _split_drain_and_barrier
    tile.TileContext._drain_split_patched = True


def build_nc(causal: bool):
    """Build the SPMD Bass program (identical on all 8 cores)."""
    _patch_tile_drain()
    from contextlib import ExitStack

    import concourse.bacc as bacc
    import concourse.tile as tile
    from concourse import mybir

    f32 = mybir.dt.float32
    f32r = mybir.dt.float32r
    bf16 = mybir.dt.bfloat16
    Exp = mybir.ActivationFunctionType.Exp
    Ln = mybir.ActivationFunctionType.Ln

    nc = bacc.Bacc("TRN2")
    # all inputs host-rearranged to partition-major so every DMA moves
    # multi-KB contiguous runs per partition (descriptor-count, not bytes,
    # is what throttles the DMA ring)
    xT_d = nc.dram_tensor("xT", [128, NQB, NDC, 512], bf16, kind="ExternalInput")
    wq_d = nc.dram_tensor("wq", [128, NDC, CH], bf16, kind="ExternalInput")
    wk_d = nc.dram_tensor("wk", [128, NDC, CH], bf16, kind="ExternalInput")
    wv_d = nc.dram_tensor("wv", [128, NDC, CH], bf16, kind="ExternalInput")
    wo_d = nc.dram_tensor("wo", [128, NCC, D], bf16, kind="ExternalInput")
    sel_d = nc.dram_tensor("sel", [4, HPC * 64], f32r, kind="ExternalInput")
    out_d = nc.dram_tensor("out", [T, D], bf16, kind="ExternalOutput")

    with tile.TileContext(nc) as tc:
        with ExitStack() as ctx:
            persist = ctx.enter_context(tc.tile_pool(name="persist", bufs=1))
            mm_ps = ctx.enter_context(
                tc.tile_pool(name="mm_ps", bufs=2, space="PSUM")
            )
            s_ps = ctx.enter_context(tc.tile_pool(name="s_ps", bufs=2, space="PSUM"))
            pv_ps = ctx.enter_context(tc.tile_pool(name="pv_ps", bufs=2, space="PSUM"))
            p_pool = ctx.enter_context(tc.tile_pool(name="p_pool", bufs=6))
            rc_pool = ctx.enter_context(tc.tile_pool(name="rc_pool", bufs=4))
            pvs_pool = ctx.enter_context(tc.tile_pool(name="pvs_pool", bufs=6))
            ob_pool = ctx.enter_context(tc.tile_pool(name="ob_pool", bufs=3))

            # ---- persistent SBUF tensors ----
            xT_sb = persist.tile([128, NQB, NDC, 512], bf16, tag="xT")  # 4 MB
            wq_sb = persist.tile([128, NDC, CH], bf16, tag="wq")       # 0.5 MB
            wk_sb = persist.tile([128, NDC, CH], bf16, tag="wk")
            wv_sb = persist.tile([128, NDC, CH], bf16, tag="wv")
            wo_sb = persist.tile([128, NCC, D], bf16, tag="wo")        # 0.5 MB
            QT_sb = persist.tile([128, NCC, T], bf16, tag="QT")        # 1 MB
            KT_sb = persist.tile([128, NCC, T], bf16, tag="KT")        # 1 MB
            V_sb = persist.tile([128, NTC, HPC, HD + 1], bf16, tag="V")
            AT_sb = persist.tile([128, NCC, T], bf16, tag="AT")
            maskm = persist.tile([128, 4, 1024], bf16, tag="maskm")
            # head-selector for the denominator broadcast matmul:
            # sel4[p, h*64+j] = (p == h), so sel4[:, h*64:(h+1)*64].T @ rc4
            # replicates rc4 row h onto 64 partitions
            sel4 = persist.tile([4, HPC * 64], f32r, tag="sel4")
            # per-(head, qb) softmax denominators, gathered cross-partition
            # by tiny SBUF->SBUF DMAs so one reciprocal covers 4 heads
            den4 = persist.tile([4, NQB, 512], f32, tag="den4")

            # ---- input DMAs, ordered so stage A(0) can start ASAP; one
            # issue per tensor/block (each dma_start serializes ~0.6us on
            # the Sync queue, so fewer+bigger is strictly better) ----
            nc.sync.dma_start(wq_sb[:], wq_d[:])
            nc.sync.dma_start(sel4[:], sel_d[:])
            nc.sync.dma_start(xT_sb[:, 0], xT_d[:, 0])
            nc.sync.dma_start(wk_sb[:], wk_d[:])
            nc.sync.dma_start(wv_sb[:], wv_d[:])
            nc.sync.dma_start(xT_sb[:, 1], xT_d[:, 1])
            nc.sync.dma_start(wo_sb[:], wo_d[:])
            nc.sync.dma_start(xT_sb[:, 2], xT_d[:, 2])
            nc.sync.dma_start(xT_sb[:, 3], xT_d[:, 3])

            # ones column of V (softmax denominator trick) — memset, a DMA
            # of this strided pattern shatters into 8192 2-byte descriptors
            nc.vector.memset(V_sb[:, :, :, HD : HD + 1], 1.0)
            # 0/1 causal masks for the four diagonal-kc offsets (i = kc-4qb);
            # used by the DVE mask path (gpsimd affine_select handles the
            # alternating halves of the stream).
            nc.vector.memset(maskm[:], 1.0)
            for i in range(4):
                nc.gpsimd.affine_select(
                    out=maskm[:, i, :].rearrange("p (a b) -> p a b", a=2),
                    in_=maskm[:, i, :].rearrange("p (a b) -> p a b", a=2),
                    compare_op=mybir.AluOpType.is_ge,
                    fill=0.0,
                    base=-128 * i,
                    pattern=[[0, 2], [1, 512]],
                    channel_multiplier=-1,
                )

            # ---- stage A: one projection "group" = one PSUM accumulation ----
            def a_group(tb, kind, j):
                tsl = slice(tb * 512, (tb + 1) * 512)
                if kind in ("Q", "K"):
                    w_sb, dst = (wq_sb, QT_sb) if kind == "Q" else (wk_sb, KT_sb)
                    ps = mm_ps.tile([128, 512], f32, tag="mmps")
                    for dc in range(NDC):
                        nc.tensor.matmul(
                            ps[:],
                            w_sb[:, dc, j * 128 : (j + 1) * 128],
                            xT_sb[:, tb, dc, :],
                            start=(dc == 0),
                            stop=(dc == NDC - 1),
                        )
                    nc.vector.tensor_copy(dst[:, j, tsl], ps[:])
                else:  # V
                    t_c = tb * 4 + j
                    ps = mm_ps.tile([128, 512], f32, tag="mmps")
                    for dc in range(NDC):
                        nc.tensor.matmul(
                            ps[:, 0:CH],
                            xT_sb[:, tb, dc, j * 128 : (j + 1) * 128],
                            wv_sb[:, dc, :],
                            start=(dc == 0),
                            stop=(dc == NDC - 1),
                        )
                    nc.vector.tensor_copy(
                        V_sb[:, t_c, :, 0:HD],
                        ps[:, 0:CH].rearrange("p (h d) -> p h d", h=HPC),
                    )

            def a_groups(tb, kinds="QKV"):
                out = []
                if "Q" in kinds:
                    out += [("A", tb, "Q", j) for j in range(NCC)]
                if "K" in kinds:
                    out += [("A", tb, "K", j) for j in range(NCC)]
                if "V" in kinds:
                    out += [("A", tb, "V", j) for j in range(4)]
                return out

            # ---- stage C: one group = one output token-chunk (2 psums,
            # one merged 256KB store) ----
            def c_group(qb, t_ci):
                t_c = qb * 4 + t_ci
                ob = ob_pool.tile([128, 1024], bf16, tag="ob")
                for eb in range(NEB):
                    esl = slice(eb * 512, (eb + 1) * 512)
                    ps = mm_ps.tile([128, 512], f32, tag="mmps")
                    for cc in range(NCC):
                        nc.tensor.matmul(
                            ps[:],
                            AT_sb[:, cc, t_c * 128 : (t_c + 1) * 128],
                            wo_sb[:, cc, esl],
                            start=(cc == 0),
                            stop=(cc == NCC - 1),
                        )
                    nc.vector.tensor_copy(ob[:, esl], ps[:])
                nc.sync.dma_start(out_d[t_c * 128 : (t_c + 1) * 128, :], ob[:])

            def emit_filler(f):
                if f[0] == "A":
                    a_group(f[1], f[2], f[3])
                elif f[0] == "N":
                    norm_head(f[1], f[2])
                else:
                    c_group(f[1], f[2])

            # ---- stage B ----
            state = {"mask_flip": False}

            def emit_unit(qb, p, kc):
                """Scores for heads (2p, 2p+1) on key-chunk kc: two
                concurrent K=64 row-tiled matmuls -> [128,1024] psum,
                one exp, optional diagonal mask."""
                qsl = slice(qb * 512, (qb + 1) * 512)
                ksl = slice(kc * 128, (kc + 1) * 128)
                s = s_ps.tile([128, 1024], f32, tag="s")
                nc.tensor.matmul(
                    s[:, 0:512],
                    KT_sb[0:64, p, ksl],
                    QT_sb[0:64, p, qsl],
                    start=True,
                    stop=True,
                )
                nc.tensor.matmul(
                    s[:, 512:1024],
                    KT_sb[64:128, p, ksl],
                    QT_sb[64:128, p, qsl],
                    start=True,
                    stop=True,
                )
                pt = p_pool.tile([128, 1024], bf16, tag="p")
                nc.scalar.activation(pt[:], s[:], Exp)
                if causal and kc >= 4 * qb:
                    i = kc - 4 * qb
                    state["mask_flip"] = not state["mask_flip"]
                    if state["mask_flip"]:
                        nc.vector.tensor_mul(pt[:], pt[:], maskm[:, i, :])
                    else:
                        nc.gpsimd.affine_select(
                            out=pt[:].rearrange("p (a b) -> p a b", a=2),
                            in_=pt[:].rearrange("p (a b) -> p a b", a=2),
                            compare_op=mybir.AluOpType.is_ge,
                            fill=0.0,
                            base=-128 * i,
                            pattern=[[0, 2], [1, 512]],
                            channel_multiplier=-1,
                        )
                return pt

            def finish_block(qb, p, pv0, pv1):
                """Evacuate the two PV accumulators of block (qb, p) and
                ship their denominator rows into den4."""
                for hoi, pv in ((0, pv0), (1, pv1)):
                    h = 2 * p + hoi
                    pvs = pvs_pool.tile([HD + 1, 512], f32, name="pvs", tag="pvs")
                    nc.vector.tensor_copy(pvs[:], pv[:])
                    nc.sync.dma_start(den4[h : h + 1, qb, :], pvs[HD : HD + 1, :])
                    state[("pvs", qb, h)] = pvs

            def emit_recip(qb):
                """Reciprocal of all 4 heads' denominators as exp(-ln(x)) on
                the ACT engine (~1.4us vs 3.3us for the DVE multi-pass
                reciprocal, and off the busier DVE queue); the per-head
                broadcast+normalize is deferred into the filler stream
                (norm_head) so the PE never waits on it."""
                ln4 = rc_pool.tile([4, 512], f32, name="ln4", tag="rc")
                nc.scalar.activation(ln4[:], den4[:, qb, :], Ln)
                rc4 = rc_pool.tile([4, 512], f32r, name="rc4", tag="rc")
                nc.scalar.activation(rc4[:], ln4[:], Exp, scale=-1.0)
                state[("rc", qb)] = rc4

            def norm_head(qb, h):
                qsl = slice(qb * 512, (qb + 1) * 512)
                dn = mm_ps.tile([64, 512], f32, name="dn", tag="mmps")
                nc.tensor.matmul(
                    dn[:],
                    sel4[:, h * 64 : (h + 1) * 64],
                    state[("rc", qb)][:],
                    start=True,
                    stop=True,
                )
                pvs = state.pop(("pvs", qb, h))
                nc.vector.tensor_mul(
                    AT_sb[(h % 2) * 64 : (h % 2 + 1) * 64, h // 2, qsl],
                    pvs[0:HD, :],
                    dn[:],
                )

            pend_pv = []
            done_blocks = {qb: 0 for qb in range(NQB)}
            flow = []  # filler queue consumed by the unit loop

            def pop_pv():
                qb, p, kc, last, pt, pv0, pv1 = pend_pv.pop(0)
                nc.tensor.matmul(
                    pv0[:],
                    V_sb[:, kc, 2 * p, :],
                    pt[:, 0:512],
                    start=(kc == 0),
                    stop=last,
                )
                nc.tensor.matmul(
                    pv1[:],
                    V_sb[:, kc, 2 * p + 1, :],
                    pt[:, 512:1024],
                    start=(kc == 0),
                    stop=last,
                )
                if last:
                    finish_block(qb, p, pv0, pv1)
                    done_blocks[qb] += 1
                    if done_blocks[qb] == NCC:
                        emit_recip(qb)
                        flow.extend(("N", qb, h) for h in range(HPC))
                        flow.extend(("C", qb, t_ci) for t_ci in range(4))

            # ---- emission schedule ----
            # A(0) head; per qb: its B units with interleaved fillers.
            # A(3) is split: Q(3) into B(2) (QT(3) gates B(3) start), K/V(3)
            # into B(3)'s early units (legal for kc<12) to fill its
            # ACT-bound bubble; C(qb) becomes ready mid-stream via pop_pv.
            for f in a_groups(0):
                emit_filler(f)

            if causal:
                section_fillers = {
                    0: a_groups(1),
                    1: a_groups(2),
                    2: a_groups(3, "Q"),
                    3: a_groups(3, "KV"),
                }
            else:
                # every query block attends to every key chunk: all
                # projections must precede stage B
                for tb in range(1, NQB):
                    for f in a_groups(tb):
                        emit_filler(f)
                section_fillers = {qb: [] for qb in range(NQB)}

            def drip(hold_c):
                """Emit one filler, skipping stage-C groups when they are
                held back to cover the tail's norm chain."""
                for idx, f in enumerate(flow):
                    if hold_c and f[0] == "C":
                        continue
                    emit_filler(flow.pop(idx))
                    return True
                return False

            for qb in range(NQB):
                nkc = 4 * (qb + 1) if causal else NKC
                flow.extend(section_fillers[qb])
                units = [(p, kc) for p in range(NCC) for kc in range(nkc)]
                hold_c = qb == NQB - 1
                for ui, (p, kc) in enumerate(units):
                    # correctness guard: this unit's K/V block must be
                    # projected already (only B(3)'s deferred K/V(3) can hit)
                    while any(
                        f[0] == "A" and f[1] <= kc // 4 for f in flow
                    ):
                        emit_filler(flow.pop(0))
                    if kc == 0:
                        state["pv"] = (
                            pv_ps.tile([HD + 1, 512], f32, name="pv0", tag="pv"),
                            pv_ps.tile([HD + 1, 512], f32, name="pv1", tag="pv"),
                        )
                    pv0, pv1 = state["pv"]
                    pt = emit_unit(qb, p, kc)
                    pend_pv.append((qb, p, kc, kc == nkc - 1, pt, pv0, pv1))
                    if len(pend_pv) > 2:
                        pop_pv()
                    drip(hold_c)
                # A fillers gate the next section; flush them now
                while any(f[0] == "A" for f in flow):
                    emit_filler(flow.pop(0))

            while pend_pv:
                pop_pv()
            while flow:
                emit_filler(flow.pop(0))

    nc.finalize()
    return nc


def make_in_maps(q_input, wq, wk, wv, wo):
    import ml_dtypes

    bf16 = ml_dtypes.bfloat16
    q_input = np.asarray(q_input, dtype=np.float32)
    wq = np.asarray(wq, dtype=np.float32)
    wk = np.asarray(wk, dtype=np.float32)
    wv = np.asarray(wv, dtype=np.float32)
    wo = np.asarray(wo, dtype=np.float32)
    scale = 1.0 / np.sqrt(np.float32(HD))
    sel = np.zeros((4, HPC * 64), np.float32)
    for h in range(HPC):
        sel[h, h * 64 : (h + 1) * 64] = 1.0

    def dmajor(w):  # [D, c] -> [128, NDC, c] partition-major
        return np.ascontiguousarray(
            w.reshape(NDC, 128, w.shape[1]).transpose(1, 0, 2)
        ).astype(bf16)

    in_maps = []
    for core in range(NCORES):
        b, g = divmod(core, GROUPS)
        G = slice(g * CH, (g + 1) * CH)
        xT = q_input[b].T  # [D, T]
        xT = np.ascontiguousarray(
            xT.reshape(NDC, 128, NQB, 512).transpose(1, 2, 0, 3)
        ).astype(bf16)  # [128, NQB, NDC, 512]
        wo_r = wo[:, G].T  # [CH, D]
        wo_r = np.ascontiguousarray(
            wo_r.reshape(NCC, 128, D).transpose(1, 0, 2)
        ).astype(bf16)  # [128, NCC, D]
        in_maps.append(
            {
                "xT": xT,
                "wq": dmajor(wq[G, :].T * scale),
                "wk": dmajor(wk[G, :].T),
                "wv": dmajor(wv[G, :].T),
                "wo": wo_r,
                "sel": sel,
            }
        )
    return in_maps


def _gather(results, bo):
    out = np.zeros((B, T, D), np.float32)
    for core in range(NCORES):
        out[core // GROUPS] += np.asarray(results[core]["out"], dtype=np.float32)
    out += np.asarray(bo, dtype=np.float32)
    return out


def _run(q_input, wq, wk, wv, wo, bo, mask, trace=False, trace_kwargs=None):
    _install_axon_ntff_hook()
    from concourse.bass_utils import run_bass_kernel_spmd

    causal = bool(np.asarray(mask).item()) if not isinstance(mask, int) else bool(mask)
    nc = build_nc(causal)
    in_maps = make_in_maps(q_input, wq, wk, wv, wo)
    res = run_bass_kernel_spmd(
        nc,
        in_maps,
        list(range(NCORES)),
        trace=trace,
        **(trace_kwargs or {}),
    )
    return _gather(res.results, bo), res


def kernel(q_input, wq, wk, wv, wo, bo, mask):
    out, _ = _run(q_input, wq, wk, wv, wo, bo, mask)
    return out


# revision 52
# speedup vs baseline: 1.2050x; 1.0196x over previous
"""Self-contained Trainium2 Bass kernel for nn_MultiHeadAttention_80942953660675.

Reference computation (B=2, T=2048, D=1024, H=16, hd=64, causal):
    q = x @ wq.T; k = x @ wk.T; v = x @ wv.T            (per-head split)
    out = softmax(q k^T / sqrt(hd) + causal_mask) v      (per batch, head)
    out = concat_heads(out) @ wo.T + bo

Sharding over 8 NeuronCores: core = (batch b, head-group g), b in {0,1},
g in {0..3}, each group = 4 heads (256 channels). wq/wk/wv column-sharded,
wo row-sharded (Megatron); host sums the 4 partial outputs per batch and
adds the bias.

Per-core kernel, v2 (all-bf16 data path, fp32 PSUM accumulation):
  - xT persistent in SBUF (4 MB bf16); weights bf16 (FWL doubles weight
    load rate vs fp32, which is what lets the packed scores run 2x)
  - scores: 2 heads packed per slot as concurrent K=64 row-tiled matmuls
    (tile_position (0,0)/(64,0) via base_partition), each [128kc x 512q]
    into adjacent PSUM banks -> one [128,1024] exp per slot
  - softmax without max subtraction (scores O(+-6)); denominator = ones
    column appended to V; normalization: per-head ACT Reciprocal on the
    [1,512] denominator row, broadcast to 64 partitions with a K=1
    f32r matmul, one DVE multiply writes AT in bf16
  - causal: above-diagonal kc tiles skipped, diagonal tiles masked after
    exp (alternating DVE mask-multiply / gpsimd affine_select)
  - global software pipeline: projection groups of block tb+1 and output-
    projection groups of block qb-1 are interleaved into stage B's
    exp-paced stream so the PE never idles; K/V projections of the last
    block are deferred into stage B(3) to fill its ACT-bound bubble
"""

import sys
import types

if "/opt/trn_rl_repo" not in sys.path:
    sys.path.insert(0, "/opt/trn_rl_repo")

import numpy as np

B, T, D = 2, 2048, 1024
H, HD = 16, 64
NCORES = 8
GROUPS = 4            # head groups (cores per batch)
HPC = H // GROUPS     # heads per core = 4
CH = HPC * HD         # channels per core = 256

NDC = D // 128        # 8   d-chunks (contraction for projections)
NCC = CH // 128       # 2   channel chunks = head pairs
NQB = T // 512        # 4   query blocks
NKC = T // 128        # 16  key chunks
NTC = T // 128        # 16  token chunks
NEB = D // 512        # 2   embed blocks (output projection)


def _install_axon_ntff_hook():
    """Inject the missing antenv.axon_hooks module so NTFF profiling
    (trace=True) works in this container. Harmless if never used."""
    if "antenv.axon_hooks" in sys.modules:
        return
    try:
        import antenv  # noqa: F401
    except ImportError:
        return
    mod = types.ModuleType("antenv.axon_hooks")
    mod._hook = None

    def _set(h):
        mod._hook = h

    def _get():
        return mod._hook

    mod.set_axon_ntff_profile_hook = _set
    mod.get_axon_ntff_profile_hook = _get
    sys.modules["antenv.axon_hooks"] = mod
    try:
        from trn_agent_boot.trn_boot import _ntff_profile_via_ctypes

        _set(_ntff_profile_via_ctypes("/opt/axon/libaxon_pjrt.so"))
    except Exception:
        pass


def _patch_tile_drain():
    """This walrus build rejects >2 embedded sync waits on a single
    instruction; TileContext's exit drain can carry many. Split the extras
    onto nop instructions placed just before the drain."""
    import concourse.tile as tile

    if getattr(tile.TileContext, "_drain_split_patched", False):
        return
    import bass_rust as _br
    from concourse.vector_clock import ScopedClock as _ScopedClock

    def _split_drain_and_barrier(self, tick_clock, wait_clock):
        nc = self.nc
        drain_inst = nc.sync.drain()
        wait_clock.add_sem_waits(
            drain_inst.ins, _ScopedClock({None: tick_clock.global_clock})
        )
        si = drain_inst.ins.sync_info
        waits = list(si.on_wait) if (si is not None and si.on_wait) else []
        if len(waits) > 1:
            bb = nc.cur_bb.bb
            si.on_wait = waits[:1]
            new_insts = []
            for w in waits[1:]:
                nop = nc.sync.nop()
                nop.ins.sync_info = _br.SyncInfo(on_wait=[w], on_update=[])
                bb.instructions.remove(nop.ins)
                new_insts.append(nop.ins)
            idx = bb.instructions.index(drain_inst.ins)
            for ni in reversed(new_insts):
                bb.instructions.insert(idx, ni)

        nc.all_engine_barrier()
        assert self.sems is not None
        popped = nc._tile_sem_poison_stack.pop()
        assert popped is self._sem_poison
        nc.clear_and_free_semaphores(list(self.sems.allocated().values()))
        nc.all_engine_barrier()

    tile.TileContext._drain_and_barrier = _split_drain_and_barrier
    tile.TileContext._drain_split_patched = True


def build_nc(causal: bool):
    """Build the SPMD Bass program (identical on all 8 cores)."""
    _patch_tile_drain()
    from contextlib import ExitStack

    import concourse.bacc as bacc
    import concourse.tile as tile
    from concourse import mybir

    f32 = mybir.dt.float32
    f32r = mybir.dt.float32r
    bf16 = mybir.dt.bfloat16
    Exp = mybir.ActivationFunctionType.Exp
    Ln = mybir.ActivationFunctionType.Ln

    nc = bacc.Bacc("TRN2")
    # all inputs host-rearranged to partition-major so every DMA moves
    # multi-KB contiguous runs per partition (descriptor-count, not bytes,
    # is what throttles the DMA ring)
    xT_d = nc.dram_tensor("xT", [128, NQB, NDC, 512], bf16, kind="ExternalInput")
    wq_d = nc.dram_tensor("wq", [128, NDC, CH], bf16, kind="ExternalInput")
    wk_d = nc.dram_tensor("wk", [128, NDC, CH], bf16, kind="ExternalInput")
    wv_d = nc.dram_tensor("wv", [128, NDC, CH], bf16, kind="ExternalInput")
    wo_d = nc.dram_tensor("wo", [128, NCC, D], bf16, kind="ExternalInput")
    sel_d = nc.dram_tensor("sel", [4, HPC * 64], f32r, kind="ExternalInput")
    out_d = nc.dram_tensor("out", [T, D], bf16, kind="ExternalOutput")

    with tile.TileContext(nc) as tc:
        with ExitStack() as ctx:
            persist = ctx.enter_context(tc.tile_pool(name="persist", bufs=1))
            mm_ps = ctx.enter_context(
                tc.tile_pool(name="mm_ps", bufs=2, space="PSUM")
            )
            s_ps = ctx.enter_context(tc.tile_pool(name="s_ps", bufs=2, space="PSUM"))
            pv_ps = ctx.enter_context(tc.tile_pool(name="pv_ps", bufs=2, space="PSUM"))
            p_pool = ctx.enter_context(tc.tile_pool(name="p_pool", bufs=6))
            rc_pool = ctx.enter_context(tc.tile_pool(name="rc_pool", bufs=4))
            pvs_pool = ctx.enter_context(tc.tile_pool(name="pvs_pool", bufs=6))
            ob_pool = ctx.enter_context(tc.tile_pool(name="ob_pool", bufs=3))

            # ---- persistent SBUF tensors ----
            xT_sb = persist.tile([128, NQB, NDC, 512], bf16, tag="xT")  # 4 MB
            wq_sb = persist.tile([128, NDC, CH], bf16, tag="wq")       # 0.5 MB
            wk_sb = persist.tile([128, NDC, CH], bf16, tag="wk")
            wv_sb = persist.tile([128, NDC, CH], bf16, tag="wv")
            wo_sb = persist.tile([128, NCC, D], bf16, tag="wo")        # 0.5 MB
            QT_sb = persist.tile([128, NCC, T], bf16, tag="QT")        # 1 MB
            KT_sb = persist.tile([128, NCC, T], bf16, tag="KT")        # 1 MB
            V_sb = persist.tile([128, NTC, HPC, HD + 1], bf16, tag="V")
            AT_sb = persist.tile([128, NCC, T], bf16, tag="AT")
            maskm = persist.tile([128, 4, 1024], bf16, tag="maskm")
            # head-selector for the denominator broadcast matmul:
            # sel4[p, h*64+j] = (p == h), so sel4[:, h*64:(h+1)*64].T @ rc4
            # replicates rc4 row h onto 64 partitions
            sel4 = persist.tile([4, HPC * 64], f32r, tag="sel4")
            # per-(head, qb) softmax denominators, gathered cross-partition
            # by tiny SBUF->SBUF DMAs so one reciprocal covers 4 heads
            den4 = persist.tile([4, NQB, 512], f32, tag="den4")

            # ---- input DMAs, ordered so stage A(0) can start ASAP; one
            # issue per tensor/block (each dma_start serializes ~0.6us on
            # the Sync queue, so fewer+bigger is strictly better) ----
            nc.sync.dma_start(wq_sb[:], wq_d[:])
            nc.sync.dma_start(sel4[:], sel_d[:])
            nc.sync.dma_start(xT_sb[:, 0], xT_d[:, 0])
            nc.sync.dma_start(wk_sb[:], wk_d[:])
            nc.sync.dma_start(wv_sb[:], wv_d[:])
            nc.sync.dma_start(xT_sb[:, 1], xT_d[:, 1])
            nc.sync.dma_start(wo_sb[:], wo_d[:])
            nc.sync.dma_start(xT_sb[:, 2], xT_d[:, 2])
            nc.sync.dma_start(xT_sb[:, 3], xT_d[:, 3])

            # ones column of V (softmax denominator trick) — memset, a DMA
            # of this strided pattern shatters into 8192 2-byte descriptors
            nc.vector.memset(V_sb[:, :, :, HD : HD + 1], 1.0)
            # 0/1 causal masks for the four diagonal-kc offsets (i = kc-4qb);
            # used by the DVE mask path (gpsimd affine_select handles the
            # alternating halves of the stream).
            nc.vector.memset(maskm[:], 1.0)
            for i in range(4):
                nc.gpsimd.affine_select(
                    out=maskm[:, i, :].rearrange("p (a b) -> p a b", a=2),
                    in_=maskm[:, i, :].rearrange("p (a b) -> p a b", a=2),
                    compare_op=mybir.AluOpType.is_ge,
                    fill=0.0,
                    base=-128 * i,
                    pattern=[[0, 2], [1, 512]],
                    channel_multiplier=-1,
                )

            # ---- stage A: one projection "group" = one PSUM accumulation ----
            def a_group(tb, kind, j):
                tsl = slice(tb * 512, (tb + 1) * 512)
                if kind in ("Q", "K"):
                    w_sb, dst = (wq_sb, QT_sb) if kind == "Q" else (wk_sb, KT_sb)
                    ps = mm_ps.tile([128, 512], f32, tag="mmps")
                    for dc in range(NDC):
                        nc.tensor.matmul(
                            ps[:],
                            w_sb[:, dc, j * 128 : (j + 1) * 128],
                            xT_sb[:, tb, dc, :],
                            start=(dc == 0),
                            stop=(dc == NDC - 1),
                        )
                    nc.vector.tensor_copy(dst[:, j, tsl], ps[:])
                else:  # V
                    t_c = tb * 4 + j
                    ps = mm_ps.tile([128, 512], f32, tag="mmps")
                    for dc in range(NDC):
                        nc.tensor.matmul(
                            ps[:, 0:CH],
                            xT_sb[:, tb, dc, j * 128 : (j + 1) * 128],
                            wv_sb[:, dc, :],
                            start=(dc == 0),
                            stop=(dc == NDC - 1),
                        )
                    nc.vector.tensor_copy(
                        V_sb[:, t_c, :, 0:HD],
                        ps[:, 0:CH].rearrange("p (h d) -> p h d", h=HPC),
                    )

            def a_groups(tb, kinds="QKV"):
                out = []
                if "Q" in kinds:
                    out += [("A", tb, "Q", j) for j in range(NCC)]
                if "K" in kinds:
                    out += [("A", tb, "K", j) for j in range(NCC)]
                if "V" in kinds:
                    out += [("A", tb, "V", j) for j in range(4)]
                return out

            # ---- stage C: one group = one output token-chunk (2 psums,
            # one merged 256KB store) ----
            def c_group(qb, t_ci):
                t_c = qb * 4 + t_ci
                ob = ob_pool.tile([128, 1024], bf16, tag="ob")
                for eb in range(NEB):
                    esl = slice(eb * 512, (eb + 1) * 512)
                    ps = mm_ps.tile([128, 512], f32, tag="mmps")
                    for cc in range(NCC):
                        nc.tensor.matmul(
                            ps[:],
                            AT_sb[:, cc, t_c * 128 : (t_c + 1) * 128],
                            wo_sb[:, cc, esl],
                            start=(cc == 0),
                            stop=(cc == NCC - 1),
                        )
                    nc.vector.tensor_copy(ob[:, esl], ps[:])
                nc.sync.dma_start(out_d[t_c * 128 : (t_c + 1) * 128, :], ob[:])

            def emit_filler(f):
                if f[0] == "A":
                    a_group(f[1], f[2], f[3])
                elif f[0] == "N":
                    norm_head(f[1], f[2])
                else:
                    c_group(f[1], f[2])

            # ---- stage B ----
            state = {"mask_flip": False}

            def emit_unit(qb, p, kc):
                """Scores for heads (2p, 2p+1) on key-chunk kc: two
                concurrent K=64 row-tiled matmuls -> [128,1024] psum,
                one exp, optional diagonal mask."""
                qsl = slice(qb * 512, (qb + 1) * 512)
                ksl = slice(kc * 128, (kc + 1) * 128)
                s = s_ps.tile([128, 1024], f32, tag="s")
                nc.tensor.matmul(
                    s[:, 0:512],
                    KT_sb[0:64, p, ksl],
                    QT_sb[0:64, p, qsl],
                    start=True,
                    stop=True,
                )
                nc.tensor.matmul(
                    s[:, 512:1024],
                    KT_sb[64:128, p, ksl],
                    QT_sb[64:128, p, qsl],
                    start=True,
                    stop=True,
                )
                pt = p_pool.tile([128, 1024], bf16, tag="p")
                nc.scalar.activation(pt[:], s[:], Exp)
                if causal and kc >= 4 * qb:
                    # all-DVE masking: gpsimd's per-instruction semaphore
                    # handling (~0.7us) puts it on the exp->mask->PV chain
                    i = kc - 4 * qb
                    nc.vector.tensor_mul(pt[:], pt[:], maskm[:, i, :])
                return pt

            def finish_block(qb, p, pv0, pv1):
                """Evacuate the two PV accumulators of block (qb, p) and
                ship their denominator rows into den4."""
                for hoi, pv in ((0, pv0), (1, pv1)):
                    h = 2 * p + hoi
                    pvs = pvs_pool.tile([HD + 1, 512], f32, name="pvs", tag="pvs")
                    nc.vector.tensor_copy(pvs[:], pv[:])
                    nc.sync.dma_start(den4[h : h + 1, qb, :], pvs[HD : HD + 1, :])
                    state[("pvs", qb, h)] = pvs

            def emit_recip(qb):
                """Reciprocal of all 4 heads' denominators as exp(-ln(x)) on
                the ACT engine (~1.4us vs 3.3us for the DVE multi-pass
                reciprocal, and off the busier DVE queue); the per-head
                broadcast+normalize is deferred into the filler stream
                (norm_head) so the PE never waits on it."""
                ln4 = rc_pool.tile([4, 512], f32, name="ln4", tag="rc")
                nc.scalar.activation(ln4[:], den4[:, qb, :], Ln)
                rc4 = rc_pool.tile([4, 512], f32r, name="rc4", tag="rc")
                nc.scalar.activation(rc4[:], ln4[:], Exp, scale=-1.0)
                state[("rc", qb)] = rc4

            def norm_head(qb, h):
                qsl = slice(qb * 512, (qb + 1) * 512)
                dn = mm_ps.tile([64, 512], f32, name="dn", tag="mmps")
                nc.tensor.matmul(
                    dn[:],
                    sel4[:, h * 64 : (h + 1) * 64],
                    state[("rc", qb)][:],
                    start=True,
                    stop=True,
                )
                pvs = state.pop(("pvs", qb, h))
                nc.vector.tensor_mul(
                    AT_sb[(h % 2) * 64 : (h % 2 + 1) * 64, h // 2, qsl],
                    pvs[0:HD, :],
                    dn[:],
                )

            pend_pv = []
            done_blocks = {qb: 0 for qb in range(NQB)}
            flow = []  # filler queue consumed by the unit loop

            def pop_pv():
                qb, p, kc, last, pt, pv0, pv1 = pend_pv.pop(0)
                nc.tensor.matmul(
                    pv0[:],
                    V_sb[:, kc, 2 * p, :],
                    pt[:, 0:512],
                    start=(kc == 0),
                    stop=last,
                )
                nc.tensor.matmul(
                    pv1[:],
                    V_sb[:, kc, 2 * p + 1, :],
                    pt[:, 512:1024],
                    start=(kc == 0),
                    stop=last,
                )
                if last:
                    finish_block(qb, p, pv0, pv1)
                    done_blocks[qb] += 1
                    if done_blocks[qb] == NCC:
                        emit_recip(qb)
                        flow.extend(("N", qb, h) for h in range(HPC))
                        flow.extend(("C", qb, t_ci) for t_ci in range(4))

            # ---- emission schedule ----
            # A(0) head; per qb: its B units with interleaved fillers.
            # A(3) is split: Q(3) into B(2) (QT(3) gates B(3) start), K/V(3)
            # into B(3)'s early units (legal for kc<12) to fill its
            # ACT-bound bubble; C(qb) becomes ready mid-stream via pop_pv.
            for f in a_groups(0):
                emit_filler(f)

            if causal:
                section_fillers = {
                    0: a_groups(1),
                    1: a_groups(2),
                    2: a_groups(3, "Q"),
                    3: a_groups(3, "KV"),
                }
            else:
                # every query block attends to every key chunk: all
                # projections must precede stage B
                for tb in range(1, NQB):
                    for f in a_groups(tb):
                        emit_filler(f)
                section_fillers = {qb: [] for qb in range(NQB)}

            def drip(hold_c):
                """Emit one filler, skipping stage-C groups when they are
                held back to cover the tail's norm chain."""
                for idx, f in enumerate(flow):
                    if hold_c and f[0] == "C":
                        continue
                    emit_filler(flow.pop(idx))
                    return True
                return False

            for qb in range(NQB):
                nkc = 4 * (qb + 1) if causal else NKC
                flow.extend(section_fillers[qb])
                units = [(p, kc) for p in range(NCC) for kc in range(nkc)]
                hold_c = qb == NQB - 1
                for ui, (p, kc) in enumerate(units):
                    # correctness guard: this unit's K/V block must be
                    # projected already (only B(3)'s deferred K/V(3) can hit)
                    while any(
                        f[0] == "A" and f[1] <= kc // 4 for f in flow
                    ):
                        emit_filler(flow.pop(0))
                    if kc == 0:
                        state["pv"] = (
                            pv_ps.tile([HD + 1, 512], f32, name="pv0", tag="pv"),
                            pv_ps.tile([HD + 1, 512], f32, name="pv1", tag="pv"),
                        )
                    pv0, pv1 = state["pv"]
                    pt = emit_unit(qb, p, kc)
                    pend_pv.append((qb, p, kc, kc == nkc - 1, pt, pv0, pv1))
                    if len(pend_pv) > 2:
                        pop_pv()
                    drip(hold_c)
                # A fillers gate the next section; flush them now
                while any(f[0] == "A" for f in flow):
                    emit_filler(flow.pop(0))

            while pend_pv:
                pop_pv()
            while flow:
                emit_filler(flow.pop(0))

    nc.finalize()
    return nc


def make_in_maps(q_input, wq, wk, wv, wo):
    import ml_dtypes

    bf16 = ml_dtypes.bfloat16
    q_input = np.asarray(q_input, dtype=np.float32)
    wq = np.asarray(wq, dtype=np.float32)
    wk = np.asarray(wk, dtype=np.float32)
    wv = np.asarray(wv, dtype=np.float32)
    wo = np.asarray(wo, dtype=np.float32)
    scale = 1.0 / np.sqrt(np.float32(HD))
    sel = np.zeros((4, HPC * 64), np.float32)
    for h in range(HPC):
        sel[h, h * 64 : (h + 1) * 64] = 1.0

    def dmajor(w):  # [D, c] -> [128, NDC, c] partition-major
        return np.ascontiguousarray(
            w.reshape(NDC, 128, w.shape[1]).transpose(1, 0, 2)
        ).astype(bf16)

    in_maps = []
    for core in range(NCORES):
        b, g = divmod(core, GROUPS)
        G = slice(g * CH, (g + 1) * CH)
        xT = q_input[b].T  # [D, T]
        xT = np.ascontiguousarray(
            xT.reshape(NDC, 128, NQB, 512).transpose(1, 2, 0, 3)
        ).astype(bf16)  # [128, NQB, NDC, 512]
        wo_r = wo[:, G].T  # [CH, D]
        wo_r = np.ascontiguousarray(
            wo_r.reshape(NCC, 128, D).transpose(1, 0, 2)
        ).astype(bf16)  # [128, NCC, D]
        in_maps.append(
            {
                "xT": xT,
                "wq": dmajor(wq[G, :].T * scale),
                "wk": dmajor(wk[G, :].T),
                "wv": dmajor(wv[G, :].T),
                "wo": wo_r,
                "sel": sel,
            }
        )
    return in_maps


def _gather(results, bo):
    out = np.zeros((B, T, D), np.float32)
    for core in range(NCORES):
        out[core // GROUPS] += np.asarray(results[core]["out"], dtype=np.float32)
    out += np.asarray(bo, dtype=np.float32)
    return out


def _run(q_input, wq, wk, wv, wo, bo, mask, trace=False, trace_kwargs=None):
    _install_axon_ntff_hook()
    from concourse.bass_utils import run_bass_kernel_spmd

    causal = bool(np.asarray(mask).item()) if not isinstance(mask, int) else bool(mask)
    nc = build_nc(causal)
    in_maps = make_in_maps(q_input, wq, wk, wv, wo)
    res = run_bass_kernel_spmd(
        nc,
        in_maps,
        list(range(NCORES)),
        trace=trace,
        **(trace_kwargs or {}),
    )
    return _gather(res.results, bo), res


def kernel(q_input, wq, wk, wv, wo, bo, mask):
    out, _ = _run(q_input, wq, wk, wv, wo, bo, mask)
    return out


# revision 53
# speedup vs baseline: 1.2159x; 1.0090x over previous
"""Self-contained Trainium2 Bass kernel for nn_MultiHeadAttention_80942953660675.

Reference computation (B=2, T=2048, D=1024, H=16, hd=64, causal):
    q = x @ wq.T; k = x @ wk.T; v = x @ wv.T            (per-head split)
    out = softmax(q k^T / sqrt(hd) + causal_mask) v      (per batch, head)
    out = concat_heads(out) @ wo.T + bo

Sharding over 8 NeuronCores: core = (batch b, head-group g), b in {0,1},
g in {0..3}, each group = 4 heads (256 channels). wq/wk/wv column-sharded,
wo row-sharded (Megatron); host sums the 4 partial outputs per batch and
adds the bias.

Per-core kernel (all-bf16 data path, fp32 PSUM accumulation):
  - all inputs host-rearranged partition-major so every DMA moves multi-KB
    contiguous runs per partition (descriptor count throttles the ring);
    xT persistent in SBUF (4 MB bf16); weights bf16 (FWL doubles the
    weight-load rate vs fp32, which is what lets packed scores run 2x)
  - scores: 2 heads packed per slot as concurrent K=64 row-tiled matmuls
    (tile_position (0,0)/(64,0) via base_partition), each [128kc x 512q]
    into adjacent PSUM banks -> one [128,1024] exp per slot
  - softmax without max subtraction (scores O(+-6)); denominator = ones
    column appended to V (free: matmul time is N cycles regardless of M);
    denominator rows gathered cross-partition into den4 by tiny SBUF->SBUF
    DMAs; reciprocal as exp(-ln(x)) on the ACT engine (vector.reciprocal
    is ~3.3us multi-pass); broadcast to 64 partitions with a K=4 f32r
    selector matmul; one DVE multiply writes AT in bf16
  - causal: above-diagonal kc tiles skipped entirely, diagonal tiles
    masked after exp with a DVE multiply by precomputed 0/1 masks
  - global software pipeline: projection psum-groups of block tb+1,
    output-projection chunks of block qb-1, and the deferred normalize
    matmuls ride a filler queue dripped one-per-unit into stage B's
    exp-paced stream so the PE never idles; K/V projections of the last
    block fill B(3)'s ACT-bound bubble, and stage-C of the second-to-last
    block is held back to cover the final norm chain before C(3)
"""

import sys
import types

if "/opt/trn_rl_repo" not in sys.path:
    sys.path.insert(0, "/opt/trn_rl_repo")

import numpy as np

B, T, D = 2, 2048, 1024
H, HD = 16, 64
NCORES = 8
GROUPS = 4            # head groups (cores per batch)
HPC = H // GROUPS     # heads per core = 4
CH = HPC * HD         # channels per core = 256

NDC = D // 128        # 8   d-chunks (contraction for projections)
NCC = CH // 128       # 2   channel chunks = head pairs
NQB = T // 512        # 4   query blocks
NKC = T // 128        # 16  key chunks
NTC = T // 128        # 16  token chunks
NEB = D // 512        # 2   embed blocks (output projection)


def _install_axon_ntff_hook():
    """Inject the missing antenv.axon_hooks module so NTFF profiling
    (trace=True) works in this container. Harmless if never used."""
    if "antenv.axon_hooks" in sys.modules:
        return
    try:
        import antenv  # noqa: F401
    except ImportError:
        return
    mod = types.ModuleType("antenv.axon_hooks")
    mod._hook = None

    def _set(h):
        mod._hook = h

    def _get():
        return mod._hook

    mod.set_axon_ntff_profile_hook = _set
    mod.get_axon_ntff_profile_hook = _get
    sys.modules["antenv.axon_hooks"] = mod
    try:
        from trn_agent_boot.trn_boot import _ntff_profile_via_ctypes

        _set(_ntff_profile_via_ctypes("/opt/axon/libaxon_pjrt.so"))
    except Exception:
        pass


def _patch_tile_drain():
    """This walrus build rejects >2 embedded sync waits on a single
    instruction; TileContext's exit drain can carry many. Split the extras
    onto nop instructions placed just before the drain."""
    import concourse.tile as tile

    if getattr(tile.TileContext, "_drain_split_patched", False):
        return
    import bass_rust as _br
    from concourse.vector_clock import ScopedClock as _ScopedClock

    def _split_drain_and_barrier(self, tick_clock, wait_clock):
        nc = self.nc
        drain_inst = nc.sync.drain()
        wait_clock.add_sem_waits(
            drain_inst.ins, _ScopedClock({None: tick_clock.global_clock})
        )
        si = drain_inst.ins.sync_info
        waits = list(si.on_wait) if (si is not None and si.on_wait) else []
        if len(waits) > 1:
            bb = nc.cur_bb.bb
            si.on_wait = waits[:1]
            new_insts = []
            for w in waits[1:]:
                nop = nc.sync.nop()
                nop.ins.sync_info = _br.SyncInfo(on_wait=[w], on_update=[])
                bb.instructions.remove(nop.ins)
                new_insts.append(nop.ins)
            idx = bb.instructions.index(drain_inst.ins)
            for ni in reversed(new_insts):
                bb.instructions.insert(idx, ni)

        nc.all_engine_barrier()
        assert self.sems is not None
        popped = nc._tile_sem_poison_stack.pop()
        assert popped is self._sem_poison
        nc.clear_and_free_semaphores(list(self.sems.allocated().values()))
        nc.all_engine_barrier()

    tile.TileContext._drain_and_barrier = _split_drain_and_barrier
    tile.TileContext._drain_split_patched = True


def build_nc(causal: bool):
    """Build the SPMD Bass program (identical on all 8 cores)."""
    _patch_tile_drain()
    from contextlib import ExitStack

    import concourse.bacc as bacc
    import concourse.tile as tile
    from concourse import mybir

    f32 = mybir.dt.float32
    f32r = mybir.dt.float32r
    bf16 = mybir.dt.bfloat16
    Exp = mybir.ActivationFunctionType.Exp
    Ln = mybir.ActivationFunctionType.Ln

    nc = bacc.Bacc("TRN2")
    # all inputs host-rearranged to partition-major so every DMA moves
    # multi-KB contiguous runs per partition (descriptor-count, not bytes,
    # is what throttles the DMA ring)
    xT_d = nc.dram_tensor("xT", [128, NQB, NDC, 512], bf16, kind="ExternalInput")
    wq_d = nc.dram_tensor("wq", [128, NDC, CH], bf16, kind="ExternalInput")
    wk_d = nc.dram_tensor("wk", [128, NDC, CH], bf16, kind="ExternalInput")
    wv_d = nc.dram_tensor("wv", [128, NDC, CH], bf16, kind="ExternalInput")
    wo_d = nc.dram_tensor("wo", [128, NCC, D], bf16, kind="ExternalInput")
    sel_d = nc.dram_tensor("sel", [4, HPC * 64], f32r, kind="ExternalInput")
    out_d = nc.dram_tensor("out", [T, D], bf16, kind="ExternalOutput")

    with tile.TileContext(nc) as tc:
        with ExitStack() as ctx:
            persist = ctx.enter_context(tc.tile_pool(name="persist", bufs=1))
            mm_ps = ctx.enter_context(
                tc.tile_pool(name="mm_ps", bufs=2, space="PSUM")
            )
            s_ps = ctx.enter_context(tc.tile_pool(name="s_ps", bufs=2, space="PSUM"))
            pv_ps = ctx.enter_context(tc.tile_pool(name="pv_ps", bufs=2, space="PSUM"))
            p_pool = ctx.enter_context(tc.tile_pool(name="p_pool", bufs=6))
            rc_pool = ctx.enter_context(tc.tile_pool(name="rc_pool", bufs=4))
            pvs_pool = ctx.enter_context(tc.tile_pool(name="pvs_pool", bufs=6))
            ob_pool = ctx.enter_context(tc.tile_pool(name="ob_pool", bufs=3))

            # ---- persistent SBUF tensors ----
            xT_sb = persist.tile([128, NQB, NDC, 512], bf16, tag="xT")  # 4 MB
            wq_sb = persist.tile([128, NDC, CH], bf16, tag="wq")       # 0.5 MB
            wk_sb = persist.tile([128, NDC, CH], bf16, tag="wk")
            wv_sb = persist.tile([128, NDC, CH], bf16, tag="wv")
            wo_sb = persist.tile([128, NCC, D], bf16, tag="wo")        # 0.5 MB
            QT_sb = persist.tile([128, NCC, T], bf16, tag="QT")        # 1 MB
            KT_sb = persist.tile([128, NCC, T], bf16, tag="KT")        # 1 MB
            V_sb = persist.tile([128, NTC, HPC, HD + 1], bf16, tag="V")
            AT_sb = persist.tile([128, NCC, T], bf16, tag="AT")
            maskm = persist.tile([128, 4, 1024], bf16, tag="maskm")
            # head-selector for the denominator broadcast matmul:
            # sel4[p, h*64+j] = (p == h), so sel4[:, h*64:(h+1)*64].T @ rc4
            # replicates rc4 row h onto 64 partitions
            sel4 = persist.tile([4, HPC * 64], f32r, tag="sel4")
            # per-(head, qb) softmax denominators, gathered cross-partition
            # by tiny SBUF->SBUF DMAs so one reciprocal covers 4 heads
            den4 = persist.tile([4, NQB, 512], f32, tag="den4")

            # ---- input DMAs, ordered so stage A(0) can start ASAP; one
            # issue per tensor/block (each dma_start serializes ~0.6us on
            # the Sync queue, so fewer+bigger is strictly better) ----
            nc.sync.dma_start(wq_sb[:], wq_d[:])
            nc.sync.dma_start(sel4[:], sel_d[:])
            nc.sync.dma_start(xT_sb[:, 0], xT_d[:, 0])
            nc.sync.dma_start(wk_sb[:], wk_d[:])
            nc.sync.dma_start(wv_sb[:], wv_d[:])
            nc.sync.dma_start(xT_sb[:, 1], xT_d[:, 1])
            nc.sync.dma_start(wo_sb[:], wo_d[:])
            nc.sync.dma_start(xT_sb[:, 2], xT_d[:, 2])
            nc.sync.dma_start(xT_sb[:, 3], xT_d[:, 3])

            # ones column of V (softmax denominator trick) — memset, a DMA
            # of this strided pattern shatters into 8192 2-byte descriptors
            nc.vector.memset(V_sb[:, :, :, HD : HD + 1], 1.0)
            # 0/1 causal masks for the four diagonal-kc offsets (i = kc-4qb);
            # used by the DVE mask path (gpsimd affine_select handles the
            # alternating halves of the stream).
            nc.vector.memset(maskm[:], 1.0)
            for i in range(4):
                nc.gpsimd.affine_select(
                    out=maskm[:, i, :].rearrange("p (a b) -> p a b", a=2),
                    in_=maskm[:, i, :].rearrange("p (a b) -> p a b", a=2),
                    compare_op=mybir.AluOpType.is_ge,
                    fill=0.0,
                    base=-128 * i,
                    pattern=[[0, 2], [1, 512]],
                    channel_multiplier=-1,
                )

            # ---- stage A: one projection "group" = one PSUM accumulation ----
            def a_group(tb, kind, j):
                tsl = slice(tb * 512, (tb + 1) * 512)
                if kind in ("Q", "K"):
                    w_sb, dst = (wq_sb, QT_sb) if kind == "Q" else (wk_sb, KT_sb)
                    ps = mm_ps.tile([128, 512], f32, tag="mmps")
                    for dc in range(NDC):
                        nc.tensor.matmul(
                            ps[:],
                            w_sb[:, dc, j * 128 : (j + 1) * 128],
                            xT_sb[:, tb, dc, :],
                            start=(dc == 0),
                            stop=(dc == NDC - 1),
                        )
                    nc.vector.tensor_copy(dst[:, j, tsl], ps[:])
                else:  # V
                    t_c = tb * 4 + j
                    ps = mm_ps.tile([128, 512], f32, tag="mmps")
                    for dc in range(NDC):
                        nc.tensor.matmul(
                            ps[:, 0:CH],
                            xT_sb[:, tb, dc, j * 128 : (j + 1) * 128],
                            wv_sb[:, dc, :],
                            start=(dc == 0),
                            stop=(dc == NDC - 1),
                        )
                    nc.vector.tensor_copy(
                        V_sb[:, t_c, :, 0:HD],
                        ps[:, 0:CH].rearrange("p (h d) -> p h d", h=HPC),
                    )

            def a_groups(tb, kinds="QKV"):
                out = []
                if "Q" in kinds:
                    out += [("A", tb, "Q", j) for j in range(NCC)]
                if "K" in kinds:
                    out += [("A", tb, "K", j) for j in range(NCC)]
                if "V" in kinds:
                    out += [("A", tb, "V", j) for j in range(4)]
                return out

            # ---- stage C: one group = one output token-chunk (2 psums,
            # one merged 256KB store) ----
            def c_group(qb, t_ci):
                t_c = qb * 4 + t_ci
                ob = ob_pool.tile([128, 1024], bf16, tag="ob")
                for eb in range(NEB):
                    esl = slice(eb * 512, (eb + 1) * 512)
                    ps = mm_ps.tile([128, 512], f32, tag="mmps")
                    for cc in range(NCC):
                        nc.tensor.matmul(
                            ps[:],
                            AT_sb[:, cc, t_c * 128 : (t_c + 1) * 128],
                            wo_sb[:, cc, esl],
                            start=(cc == 0),
                            stop=(cc == NCC - 1),
                        )
                    nc.vector.tensor_copy(ob[:, esl], ps[:])
                nc.sync.dma_start(out_d[t_c * 128 : (t_c + 1) * 128, :], ob[:])

            def emit_filler(f):
                if f[0] == "A":
                    a_group(f[1], f[2], f[3])
                elif f[0] == "N":
                    norm_head(f[1], f[2])
                else:
                    c_group(f[1], f[2])

            # ---- stage B ----
            state = {"mask_flip": False}

            def emit_unit(qb, p, kc):
                """Scores for heads (2p, 2p+1) on key-chunk kc: two
                concurrent K=64 row-tiled matmuls -> [128,1024] psum,
                one exp, optional diagonal mask."""
                qsl = slice(qb * 512, (qb + 1) * 512)
                ksl = slice(kc * 128, (kc + 1) * 128)
                s = s_ps.tile([128, 1024], f32, tag="s")
                nc.tensor.matmul(
                    s[:, 0:512],
                    KT_sb[0:64, p, ksl],
                    QT_sb[0:64, p, qsl],
                    start=True,
                    stop=True,
                )
                nc.tensor.matmul(
                    s[:, 512:1024],
                    KT_sb[64:128, p, ksl],
                    QT_sb[64:128, p, qsl],
                    start=True,
                    stop=True,
                )
                pt = p_pool.tile([128, 1024], bf16, tag="p")
                nc.scalar.activation(pt[:], s[:], Exp)
                if causal and kc >= 4 * qb:
                    # all-DVE masking: gpsimd's per-instruction semaphore
                    # handling (~0.7us) puts it on the exp->mask->PV chain
                    i = kc - 4 * qb
                    nc.vector.tensor_mul(pt[:], pt[:], maskm[:, i, :])
                return pt

            def finish_block(qb, p, pv0, pv1):
                """Evacuate the two PV accumulators of block (qb, p) and
                ship their denominator rows into den4."""
                for hoi, pv in ((0, pv0), (1, pv1)):
                    h = 2 * p + hoi
                    pvs = pvs_pool.tile([HD + 1, 512], f32, name="pvs", tag="pvs")
                    nc.vector.tensor_copy(pvs[:], pv[:])
                    nc.sync.dma_start(den4[h : h + 1, qb, :], pvs[HD : HD + 1, :])
                    state[("pvs", qb, h)] = pvs

            def emit_recip(qb):
                """Reciprocal of all 4 heads' denominators as exp(-ln(x)) on
                the ACT engine (~1.4us vs 3.3us for the DVE multi-pass
                reciprocal, and off the busier DVE queue); the per-head
                broadcast+normalize is deferred into the filler stream
                (norm_head) so the PE never waits on it."""
                ln4 = rc_pool.tile([4, 512], f32, name="ln4", tag="rc")
                nc.scalar.activation(ln4[:], den4[:, qb, :], Ln)
                rc4 = rc_pool.tile([4, 512], f32r, name="rc4", tag="rc")
                nc.scalar.activation(rc4[:], ln4[:], Exp, scale=-1.0)
                state[("rc", qb)] = rc4

            def norm_head(qb, h):
                qsl = slice(qb * 512, (qb + 1) * 512)
                dn = mm_ps.tile([64, 512], f32, name="dn", tag="mmps")
                nc.tensor.matmul(
                    dn[:],
                    sel4[:, h * 64 : (h + 1) * 64],
                    state[("rc", qb)][:],
                    start=True,
                    stop=True,
                )
                pvs = state.pop(("pvs", qb, h))
                nc.vector.tensor_mul(
                    AT_sb[(h % 2) * 64 : (h % 2 + 1) * 64, h // 2, qsl],
                    pvs[0:HD, :],
                    dn[:],
                )

            pend_pv = []
            done_blocks = {qb: 0 for qb in range(NQB)}
            flow = []  # filler queue consumed by the unit loop

            def pop_pv():
                qb, p, kc, last, pt, pv0, pv1 = pend_pv.pop(0)
                nc.tensor.matmul(
                    pv0[:],
                    V_sb[:, kc, 2 * p, :],
                    pt[:, 0:512],
                    start=(kc == 0),
                    stop=last,
                )
                nc.tensor.matmul(
                    pv1[:],
                    V_sb[:, kc, 2 * p + 1, :],
                    pt[:, 512:1024],
                    start=(kc == 0),
                    stop=last,
                )
                if last:
                    finish_block(qb, p, pv0, pv1)
                    done_blocks[qb] += 1
                    if done_blocks[qb] == NCC:
                        emit_recip(qb)
                        flow.extend(("N", qb, h) for h in range(HPC))
                        flow.extend(("C", qb, t_ci) for t_ci in range(4))

            # ---- emission schedule ----
            # A(0) head; per qb: its B units with interleaved fillers.
            # A(3) is split: Q(3) into B(2) (QT(3) gates B(3) start), K/V(3)
            # into B(3)'s early units (legal for kc<12) to fill its
            # ACT-bound bubble; C(qb) becomes ready mid-stream via pop_pv.
            for f in a_groups(0):
                emit_filler(f)

            if causal:
                section_fillers = {
                    0: a_groups(1),
                    1: a_groups(2),
                    2: a_groups(3, "Q"),
                    3: a_groups(3, "KV"),
                }
            else:
                # every query block attends to every key chunk: all
                # projections must precede stage B
                for tb in range(1, NQB):
                    for f in a_groups(tb):
                        emit_filler(f)
                section_fillers = {qb: [] for qb in range(NQB)}

            def drip(hold_c):
                """Emit one filler, skipping stage-C groups when they are
                held back to cover the tail's norm chain."""
                for idx, f in enumerate(flow):
                    if hold_c and f[0] == "C":
                        continue
                    emit_filler(flow.pop(idx))
                    return True
                return False

            for qb in range(NQB):
                nkc = 4 * (qb + 1) if causal else NKC
                flow.extend(section_fillers[qb])
                units = [(p, kc) for p in range(NCC) for kc in range(nkc)]
                hold_c = qb == NQB - 1
                for ui, (p, kc) in enumerate(units):
                    # correctness guard: this unit's K/V block must be
                    # projected already (only B(3)'s deferred K/V(3) can hit)
                    while any(
                        f[0] == "A" and f[1] <= kc // 4 for f in flow
                    ):
                        emit_filler(flow.pop(0))
                    if kc == 0:
                        state["pv"] = (
                            pv_ps.tile([HD + 1, 512], f32, name="pv0", tag="pv"),
                            pv_ps.tile([HD + 1, 512], f32, name="pv1", tag="pv"),
                        )
                    pv0, pv1 = state["pv"]
                    pt = emit_unit(qb, p, kc)
                    pend_pv.append((qb, p, kc, kc == nkc - 1, pt, pv0, pv1))
                    if len(pend_pv) > 2:
                        pop_pv()
                    drip(hold_c)
                # A fillers gate the next section; flush them now
                while any(f[0] == "A" for f in flow):
                    emit_filler(flow.pop(0))

            while pend_pv:
                pop_pv()
            while flow:
                emit_filler(flow.pop(0))

    nc.finalize()
    return nc


def make_in_maps(q_input, wq, wk, wv, wo):
    import ml_dtypes

    bf16 = ml_dtypes.bfloat16
    q_input = np.asarray(q_input, dtype=np.float32)
    wq = np.asarray(wq, dtype=np.float32)
    wk = np.asarray(wk, dtype=np.float32)
    wv = np.asarray(wv, dtype=np.float32)
    wo = np.asarray(wo, dtype=np.float32)
    scale = 1.0 / np.sqrt(np.float32(HD))
    sel = np.zeros((4, HPC * 64), np.float32)
    for h in range(HPC):
        sel[h, h * 64 : (h + 1) * 64] = 1.0

    def dmajor(w):  # [D, c] -> [128, NDC, c] partition-major
        return np.ascontiguousarray(
            w.reshape(NDC, 128, w.shape[1]).transpose(1, 0, 2)
        ).astype(bf16)

    in_maps = []
    for core in range(NCORES):
        b, g = divmod(core, GROUPS)
        G = slice(g * CH, (g + 1) * CH)
        xT = q_input[b].T  # [D, T]
        xT = np.ascontiguousarray(
            xT.reshape(NDC, 128, NQB, 512).transpose(1, 2, 0, 3)
        ).astype(bf16)  # [128, NQB, NDC, 512]
        wo_r = wo[:, G].T  # [CH, D]
        wo_r = np.ascontiguousarray(
            wo_r.reshape(NCC, 128, D).transpose(1, 0, 2)
        ).astype(bf16)  # [128, NCC, D]
        in_maps.append(
            {
                "xT": xT,
                "wq": dmajor(wq[G, :].T * scale),
                "wk": dmajor(wk[G, :].T),
                "wv": dmajor(wv[G, :].T),
                "wo": wo_r,
                "sel": sel,
            }
        )
    return in_maps


def _gather(results, bo):
    out = np.zeros((B, T, D), np.float32)
    for core in range(NCORES):
        out[core // GROUPS] += np.asarray(results[core]["out"], dtype=np.float32)
    out += np.asarray(bo, dtype=np.float32)
    return out


def _run(q_input, wq, wk, wv, wo, bo, mask, trace=False, trace_kwargs=None):
    _install_axon_ntff_hook()
    from concourse.bass_utils import run_bass_kernel_spmd

    causal = bool(np.asarray(mask).item()) if not isinstance(mask, int) else bool(mask)
    nc = build_nc(causal)
    in_maps = make_in_maps(q_input, wq, wk, wv, wo)
    res = run_bass_kernel_spmd(
        nc,
        in_maps,
        list(range(NCORES)),
        trace=trace,
        **(trace_kwargs or {}),
    )
    return _gather(res.results, bo), res


def kernel(q_input, wq, wk, wv, wo, bo, mask):
    out, _ = _run(q_input, wq, wk, wv, wo, bo, mask)
    return out
